# revision 69
# baseline (speedup 1.0000x reference)
"""Trainium2 Bass kernel for nn_CCNN (banded continuous-kernel conv).

Math: the reference builds a full (B,L,L) pairwise tensor, runs a tiny
scalar->8x8-matrix MLP on every (i,j) pair, masks to the band
j in [i-5, i-1], and contracts:  x_new[b,i,:] = x[b,i,:] @ sum_j kv[b,i,j].
Only the 5 sub-diagonals survive the band mask, so we evaluate the MLP
only on the 5 offsets o=1..5 per row:  dt_o = t_i - t_{i-o}.

Layout on device (per core, R=256 rows of the flattened (B*L) row axis):
  - hidden dims on partitions, rows on the free dim (256 columns)
  - all 5 offsets are batched into one matmul chain via block-diagonal
    weights. The o axis splits 3+2 (h2 = 5*32 = 160 > 128 partitions);
    the B-half (offsets 3..4) lives at base partition 64 (PE quadrant
    rule: lhsT/rhs base in {0,32,64} and equal).
  - h3 rows 48:64 are memset to 1.0 and pair with nmask in mask[48] and
    B4 in W4pad[48] to fold the +B4*nmask bias term into the W4 matmul.
  - the per-row x contraction uses selection-matrix matmuls:
      xe[(c,d), r] = x[c, r]     (partition broadcast via matmul)
      prod = Msum * xe           (elementwise)
      x_new[d, r] = sum_c prod[(c,d), r]   (selection matmul)
  - matmuls run in fp32r (TF32-like, 11-bit mantissa, 4x faster than
    fp32 on the PE): weights are pre-rounded on the host, activations
    are rounded by their producing instruction writing an fp32r tile.
    End-to-end output error vs the fp32 reference is ~3e-4 of scale.
  - the two layers' MLP pipelines are independent (both depend only on
    dt); their instructions are interleaved so the PE stays dense.
"""

import numpy as np

F = 2
KW = 5  # band width (kernel size)
CIN = 8
COUT = 8
H1, H2, H3 = 16, 32, 16
NT = 100  # n_types
B, L = 4, 512
NCORES = 8
R = (B * L) // NCORES  # 256 rows per core

# offsets 0..2 are the A-half (base partition 0), 3..4 the B-half (base 64)
OA, OB = 3, 2

TRACE = False
LAST_RESULTS = None
F32R_ENABLED = True  # fp32r (TF32-like) matmuls; flip False for full fp32

_cache = {}


def _round_f32r(x):
    """Round-to-nearest keeping 11 mantissa bits (hardware fp32r format)."""
    if not F32R_ENABLED:
        return np.ascontiguousarray(x, np.float32)
    b = np.ascontiguousarray(x, np.float32).view(np.uint32)
    b = (b + np.uint32(0x800)) & np.uint32(0xFFFFF000)
    return b.view(np.float32)


def _layer_weight_items(f):
    return [
        (f"W1pad{f}", KW, 96, 0),          # cols 0:48 = W1A blkdiag, 64:96 = W1B
        (f"W2A{f}", OA * H1, OA * H2, 0),      # (48, 96)
        (f"W2B{f}", OB * H1, OB * H2, 64),     # (32, 64) @ base 64
        (f"W3A{f}", OA * H2, OA * H3, 0),      # (96, 48)
        (f"W3B{f}", OB * H2, OB * H3, 0),      # (64, 32)
        (f"W4pad{f}", 96, CIN * COUT, 0),      # 0:48 W4A, 48 B4, 64:96 W4B
    ]


def _alloc_cols(items):
    cols = {}
    col = 0
    for name, p, w, base in items:
        cols[name] = (p, col, w, base)
        col += w
    return cols, col


def _wpack_layout():
    """Weight packs (fp32r): all matmul stationary operands.

    Split into three DMA units ordered by when the pipeline needs them:
    layer-0 weights (gate the first matmuls), layer-1 weights, selection
    matrices. W2B sits at base partition 64 (its rhs h1[64:96] is at base
    64 and the PE requires equal lhsT/rhs base partitions).
    """
    wsel = [
        ("embX", NT, CIN * COUT, 0),
        ("SelX", CIN * COUT, CIN * COUT, 0),
        ("sel8", CIN * COUT, COUT, 0),
    ]
    return (
        _alloc_cols(_layer_weight_items(0)),
        _alloc_cols(_layer_weight_items(1)),
        _alloc_cols(wsel),
    )


def _bpack_layout():
    """Bias pack (fp32): per-partition bias columns for the ACT/DVE stages."""
    items = []
    for f in range(F):
        items += [
            (f"B1pad{f}", 96, 1, 0),
            (f"B2A{f}", OA * H2, 1, 0),
            (f"B2B{f}", OB * H2, 1, 0),
            (f"B3A{f}", OA * H3, 1, 0),
            (f"B3B{f}", OB * H3, 1, 0),
        ]
    cols = {}
    col = 0
    for name, p, w, base in items:
        cols[name] = (p, col, w, base)
        col += w
    return cols, col


def _build_pack_arrays(emb, W1, B1, W2, B2, W3, B3, W4, B4):
    (c0, W0), (c1, W1c), (cs, Ws) = _wpack_layout()
    bcols, bW = _bpack_layout()
    wl0 = np.zeros((128, W0), np.float32)
    wl1 = np.zeros((128, W1c), np.float32)
    wsel = np.zeros((128, Ws), np.float32)
    bpack = np.zeros((128, bW), np.float32)

    def put(pack, cols, name, arr):
        p, col, w, base = cols[name]
        assert arr.shape == (p, w), (name, arr.shape, (p, w))
        pack[base : base + p, col : col + w] = arr

    put(wsel, cs, "embX", np.repeat(emb.astype(np.float32), COUT, axis=1))
    selx = np.zeros((CIN * COUT, CIN * COUT), np.float32)
    for cp in range(CIN):
        for dp in range(COUT):
            for d in range(COUT):
                selx[cp * COUT + dp, dp * COUT + d] = 1.0
    put(wsel, cs, "SelX", selx)
    put(wsel, cs, "sel8", np.tile(np.eye(COUT, dtype=np.float32), (CIN, 1)))

    for f in range(F):
        wpack, wcols = (wl0, c0) if f == 0 else (wl1, c1)
        w1f = W1[f].reshape(H1).astype(np.float32)
        w2f = W2[f].astype(np.float32)
        w3f = W3[f].astype(np.float32)
        w4f = W4[f].astype(np.float32)

        w1p = np.zeros((KW, 96), np.float32)
        for o in range(OA):
            w1p[o, o * H1 : (o + 1) * H1] = w1f
        for o in range(OB):
            w1p[OA + o, 64 + o * H1 : 64 + (o + 1) * H1] = w1f
        put(wpack, wcols, f"W1pad{f}", w1p)
        b1p = np.zeros((96, 1), np.float32)
        b1p[0:48, 0] = np.tile(B1[f], OA)
        b1p[64:96, 0] = np.tile(B1[f], OB)
        put(bpack, bcols, f"B1pad{f}", b1p)

        w2a = np.zeros((OA * H1, OA * H2), np.float32)
        for o in range(OA):
            w2a[o * H1 : (o + 1) * H1, o * H2 : (o + 1) * H2] = w2f
        put(wpack, wcols, f"W2A{f}", w2a)
        put(bpack, bcols, f"B2A{f}", np.tile(B2[f], OA)[:, None].astype(np.float32))
        w2b = np.zeros((OB * H1, OB * H2), np.float32)
        for o in range(OB):
            w2b[o * H1 : (o + 1) * H1, o * H2 : (o + 1) * H2] = w2f
        put(wpack, wcols, f"W2B{f}", w2b)
        put(bpack, bcols, f"B2B{f}", np.tile(B2[f], OB)[:, None].astype(np.float32))

        w3a = np.zeros((OA * H2, OA * H3), np.float32)
        for o in range(OA):
            w3a[o * H2 : (o + 1) * H2, o * H3 : (o + 1) * H3] = w3f
        put(wpack, wcols, f"W3A{f}", w3a)
        put(bpack, bcols, f"B3A{f}", np.tile(B3[f], OA)[:, None].astype(np.float32))
        w3b = np.zeros((OB * H2, OB * H3), np.float32)
        for o in range(OB):
            w3b[o * H2 : (o + 1) * H2, o * H3 : (o + 1) * H3] = w3f
        put(wpack, wcols, f"W3B{f}", w3b)
        put(bpack, bcols, f"B3B{f}", np.tile(B3[f], OB)[:, None].astype(np.float32))

        w4p = np.zeros((96, CIN * COUT), np.float32)
        w4p[0:48] = np.tile(w4f, (OA, 1))
        w4p[48] = B4[f]
        w4p[64:96] = np.tile(w4f, (OB, 1))
        put(wpack, wcols, f"W4pad{f}", w4p)

    return _round_f32r(wl0), _round_f32r(wl1), _round_f32r(wsel), bpack


def _build_nc():
    import concourse.bacc as bacc
    import concourse.mybir as mybir
    from concourse.tile import TileContext

    F32 = mybir.dt.float32
    F32R = mybir.dt.float32r if F32R_ENABLED else mybir.dt.float32
    RELU = mybir.ActivationFunctionType.Relu
    ADD = mybir.AluOpType.add
    MAX = mybir.AluOpType.max

    (c0, W0), (c1, W1c), (cs, Ws) = _wpack_layout()
    bcols, bW = _bpack_layout()

    nc = bacc.Bacc("TRN2", debug=False)
    # tvec frame (5, 512): cols 0:256 = t_i, cols 256:512 = t_{i-1-o}
    tvec_d = nc.dram_tensor("tvec", (KW, 2 * R), F32, kind="ExternalInput")
    wl0_d = nc.dram_tensor("wl0", (128, W0), F32R, kind="ExternalInput")
    wl1_d = nc.dram_tensor("wl1", (128, W1c), F32R, kind="ExternalInput")
    wsel_d = nc.dram_tensor("wsel", (128, Ws), F32R, kind="ExternalInput")
    bpack_d = nc.dram_tensor("bpack", (128, bW), F32, kind="ExternalInput")
    onehot_d = nc.dram_tensor("onehot", (NT, R), F32R, kind="ExternalInput")
    # mask96 rows: 0:48 = offsets 0..2 (x16), 48 = nmask, 49:64 = 0,
    # 64:96 = offsets 3..4 (x16)
    mask_d = nc.dram_tensor("mask96", (96, R), F32, kind="ExternalInput")
    out_d = nc.dram_tensor("out", (CIN, R), F32, kind="ExternalOutput")

    with TileContext(nc) as tc:
        with (
            tc.tile_pool(name="const", bufs=1) as cpool,
            tc.tile_pool(name="work", bufs=2) as wpool,
            tc.tile_pool(name="psum", bufs=2, space="PSUM") as ppool,
        ):
            # DMA order matters: the HWDGE transfers serialize in dispatch
            # order, so the chain-gating tensors (tvec, layer-0 weights,
            # layer-1 weights) go first on SP; mask/bias ride the SWDGE
            # (Pool) queue; onehot goes on the ACT queue.
            # warm the ACT piecewise-poly table before any real activation:
            # this dummy relu has no DMA deps, so the auto-inserted
            # LoadActFuncSet (1.3us) runs during the DMA phase
            warm = cpool.tile([1, 1], F32, tag="warm")
            nc.vector.memset(warm, 0.0)
            nc.scalar.activation(out=warm, in_=warm, func=RELU)

            tvt = cpool.tile([KW, 2 * R], F32, tag="tvec")
            nc.sync.dma_start(out=tvt, in_=tvec_d.ap())
            wl0 = cpool.tile([128, W0], F32R, tag="wl0")
            nc.sync.dma_start(out=wl0[0:96, :], in_=wl0_d.ap()[0:96, :])
            wl1 = cpool.tile([128, W1c], F32R, tag="wl1")
            nc.sync.dma_start(out=wl1[0:96, :], in_=wl1_d.ap()[0:96, :])
            wsel = cpool.tile([128, Ws], F32R, tag="wsel")
            nc.sync.dma_start(out=wsel[0:NT, :], in_=wsel_d.ap()[0:NT, :])
            onehot = cpool.tile([NT, R], F32R, tag="onehot")
            nc.sync.dma_start(out=onehot, in_=onehot_d.ap())
            bpack = cpool.tile([128, bW], F32, tag="bpack")
            nc.gpsimd.dma_start(out=bpack[0:96, :], in_=bpack_d.ap()[0:96, :])
            mask96 = cpool.tile([96, R], F32, tag="mask96")
            nc.gpsimd.dma_start(out=mask96, in_=mask_d.ap())

            def wslice(name):
                for pk, cols in ((wl0, c0), (wl1, c1), (wsel, cs)):
                    if name in cols:
                        p, col, w, base = cols[name]
                        return pk[base : base + p, col : col + w]
                raise KeyError(name)

            def bslice(name):
                p, col, w, base = bcols[name]
                return bpack[base : base + p, col : col + w]

            # dt[o, r] = t_i - t_{i-1-o} (garbage where masked; masked later)
            dt = wpool.tile([KW, R], F32R, tag="dt")
            nc.vector.tensor_sub(out=dt, in0=tvt[:, 0:R], in1=tvt[:, R : 2 * R])

            # ---- the 5-offset MLPs of both layers, interleaved stage by
            # stage so the PE runs dense (they only depend on dt) ----
            h1ps, h1, h2psA, h2psB, h2A, h2B = {}, {}, {}, {}, {}, {}
            h3ps, h3, h3m, msum = {}, {}, {}, {}

            for f in range(F):
                h1ps[f] = ppool.tile([96, R], F32, tag="mm", bufs=5, name=f"h1ps{f}")
                nc.tensor.matmul(h1ps[f], wslice(f"W1pad{f}"), dt, start=True, stop=True)
            for f in range(F):
                # relu+bias split column-wise across ACT and DVE (both
                # idle here): halves finish in ~250ns instead of 400ns
                h1[f] = wpool.tile([96, R], F32R, tag="h1", name=f"h1_{f}")
                half = R // 2
                nc.scalar.activation(out=h1[f][:, 0:half], in_=h1ps[f][:, 0:half], func=RELU, bias=bslice(f"B1pad{f}"))
                nc.vector.tensor_scalar(h1[f][:, half:R], h1ps[f][:, half:R], bslice(f"B1pad{f}"), 0.0, ADD, MAX)
            for f in range(F):
                h2psA[f] = ppool.tile([OA * H2, R], F32, tag="mm", bufs=5, name=f"h2psA{f}")
                nc.tensor.matmul(h2psA[f], wslice(f"W2A{f}"), h1[f][0 : OA * H1, :], start=True, stop=True)
                h2psB[f] = ppool.tile([OB * H2, R], F32, tag="mm", bufs=5, name=f"h2psB{f}")
                nc.tensor.matmul(h2psB[f], wslice(f"W2B{f}"), h1[f][64 : 64 + OB * H1, :], start=True, stop=True)
            for f in range(F):
                h2A[f] = wpool.tile([OA * H2, R], F32R, tag="h2A", name=f"h2A_{f}")
                nc.scalar.activation(out=h2A[f], in_=h2psA[f], func=RELU, bias=bslice(f"B2A{f}"))
                h2B[f] = wpool.tile([OB * H2, R], F32R, tag="h2B", name=f"h2B_{f}")
                nc.vector.tensor_scalar(h2B[f], h2psB[f], bslice(f"B2B{f}"), 0.0, ADD, MAX)
            h3psB = {}
            for f in range(F):
                h3ps[f] = ppool.tile([OA * H3, R], F32, tag="mm", bufs=5, name=f"h3ps{f}")
                nc.tensor.matmul(h3ps[f], wslice(f"W3A{f}"), h2A[f], start=True, stop=True)
                h3psB[f] = ppool.tile([OB * H3, R], F32, tag="mm", bufs=5, name=f"h3psB{f}")
                nc.tensor.matmul(h3psB[f], wslice(f"W3B{f}"), h2B[f], start=True, stop=True)
            for f in range(F):
                # rows 48:64 become 1.0 (row 48 pairs with nmask/B4); memset
                # [32:64] runs before act3A overwrites [0:48]
                h3[f] = wpool.tile([96, R], F32, tag="h3", name=f"h3_{f}")
                nc.gpsimd.memset(h3[f][32:64, :], 1.0)

            # xe[(c,d), r] = x0[c, r] = (emb gather), partition-broadcast
            # over d. Emitted here so in PE program order it lands in the
            # gap between the mm3 and mm4 stages (it depends on the
            # late-arriving wsel + onehot DMAs).
            xe_ps = ppool.tile([CIN * COUT, R], F32, tag="xe_ps", bufs=1)
            nc.tensor.matmul(xe_ps, wslice("embX"), onehot, start=True, stop=True)
            xe = wpool.tile([CIN * COUT, R], F32, tag="xe")
            nc.scalar.copy(out=xe, in_=xe_ps)

            # h3 stage + mask: DVE program order is layer-0's critical chain
            # (ts3B_0 -> h3m_0), with layer-1's ts3B slotted after; layer-1's
            # mask-mul goes to GpSimd so DVE stays free for prod0.
            nc.scalar.activation(out=h3[0][0 : OA * H3, :], in_=h3ps[0], func=RELU, bias=bslice("B3A0"))
            nc.vector.tensor_scalar(h3[0][64 : 64 + OB * H3, :], h3psB[0], bslice("B3B0"), 0.0, ADD, MAX)
            h3m[0] = wpool.tile([96, R], F32R, tag="h3m", name="h3m_0")
            nc.vector.tensor_mul(out=h3m[0], in0=h3[0], in1=mask96)
            msum[0] = ppool.tile([CIN * COUT, R], F32, tag="msum", bufs=2, name="msum0")
            nc.tensor.matmul(msum[0], wslice("W4pad0"), h3m[0], start=True, stop=True)

            nc.scalar.activation(out=h3[1][0 : OA * H3, :], in_=h3ps[1], func=RELU, bias=bslice("B3A1"))
            # layer-1 B-half on GpSimd (fp32 out): keeps DVE clear for the
            # h3m_0 -> prod0 critical chain
            nc.gpsimd.tensor_scalar(h3[1][64 : 64 + OB * H3, :], h3psB[1], bslice("B3B1"), 0.0, ADD, MAX)
            # layer 1's mask-mul follows h3m_0 on DVE: it fits in the
            # shadow of mm4_0, keeping msum1 in the fast fp32r path
            h3m[1] = wpool.tile([96, R], F32R, tag="h3m", name="h3m_1")
            nc.vector.tensor_mul(out=h3m[1], in0=h3[1], in1=mask96)

            # ---- serial x-contraction tail ----
            prod0 = wpool.tile([CIN * COUT, R], F32R, tag="prod")
            nc.vector.tensor_mul(out=prod0, in0=msum[0], in1=xe)
            xe_ps2 = ppool.tile([CIN * COUT, R], F32, tag="xe_ps", bufs=1)
            nc.tensor.matmul(xe_ps2, wslice("SelX"), prod0, start=True, stop=True)
            # msum1 emitted after selx: prod1 only needs it once selx is
            # done, and the PE slots it into selx's dependency-wait gap
            msum[1] = ppool.tile([CIN * COUT, R], F32, tag="msum", bufs=2, name="msum1")
            nc.tensor.matmul(msum[1], wslice("W4pad1"), h3m[1], start=True, stop=True)
            # msum1 moves PSUM->SBUF on the idle ACT engine, off the
            # critical chain, so prod1 can read the SelX result (xe_ps2)
            # straight from PSUM — no copy on the critical path
            msum1s = wpool.tile([CIN * COUT, R], F32, tag="xe")
            nc.scalar.copy(out=msum1s, in_=msum[1])

            prod1 = wpool.tile([CIN * COUT, R], F32R, tag="prod")
            nc.vector.tensor_mul(out=prod1, in0=xe_ps2, in1=msum1s)
            out_ps = ppool.tile([CIN, R], F32, tag="xe_ps", bufs=1)
            nc.tensor.matmul(out_ps, wslice("sel8"), prod1, start=True, stop=True)
            xout = wpool.tile([CIN, R], F32, tag="xout")
            nc.vector.tensor_copy(out=xout, in_=out_ps)
            nc.sync.dma_start(out=out_d.ap(), in_=xout)

    nc.finalize()
    return nc


def _per_core_inputs(times, features, core):
    rows = np.arange(core * R, (core + 1) * R)
    b = rows // L
    i = rows % L

    tcur = times[b, i].astype(np.float32)
    tc5 = np.tile(tcur, (KW, 1))
    tp5 = np.zeros((KW, R), np.float32)
    mask = np.zeros((KW, R), np.float32)
    for o in range(1, KW + 1):
        valid = i >= o
        tp5[o - 1, valid] = times[b[valid], i[valid] - o]
        mask[o - 1, valid] = 1.0
    mask96 = np.zeros((96, R), np.float32)
    mask96[0 : OA * H3] = np.repeat(mask[:OA], H3, axis=0)  # partition (o*16+h)
    mask96[48] = mask.sum(axis=0)  # nmask row (pairs with ones/B4 at 48)
    mask96[64 : 64 + OB * H3] = np.repeat(mask[OA:], H3, axis=0)
    tvec = np.ascontiguousarray(np.concatenate([tc5, tp5], axis=1))  # (5, 512)

    feat = features[b, i].astype(np.int64)
    onehot = (feat[None, :] == np.arange(NT)[:, None]).astype(np.float32)
    return tvec, mask96, onehot


def kernel(times, features, emb, W1, B1, W2, B2, W3, B3, W4, B4):
    global LAST_RESULTS
    from concourse.bass_utils import run_bass_kernel_spmd

    times = np.asarray(times, dtype=np.float32)
    features = np.asarray(features)
    emb = np.asarray(emb, dtype=np.float32)
    W1, B1 = np.asarray(W1, np.float32), np.asarray(B1, np.float32)
    W2, B2 = np.asarray(W2, np.float32), np.asarray(B2, np.float32)
    W3, B3 = np.asarray(W3, np.float32), np.asarray(B3, np.float32)
    W4, B4 = np.asarray(W4, np.float32), np.asarray(B4, np.float32)

    if "nc" not in _cache:
        _cache["nc"] = _build_nc()
    nc = _cache["nc"]

    wl0, wl1, wsel, bpack = _build_pack_arrays(emb, W1, B1, W2, B2, W3, B3, W4, B4)

    in_maps = []
    for core in range(NCORES):
        tvec, mask96, onehot = _per_core_inputs(times, features, core)
        in_maps.append(
            {
                "tvec": tvec,
                "wl0": wl0,
                "wl1": wl1,
                "wsel": wsel,
                "bpack": bpack,
                "onehot": onehot,
                "mask96": mask96,
            }
        )

    res = run_bass_kernel_spmd(nc, in_maps, list(range(NCORES)), trace=TRACE)
    LAST_RESULTS = res

    out = np.zeros((B * L, CIN), np.float32)
    for core in range(NCORES):
        out[core * R : (core + 1) * R, :] = res.results[core]["out"].T
    return out.reshape(B, L, CIN)
